# revision 22
# baseline (speedup 1.0000x reference)
"""STEBitLinear Trainium2 kernel.

y[b,s,o] = sum_i x[b,s,i] * sign(w[o,i]) * scale[o, i//128]

Strategy: data-parallel over the flattened (b,s) dim across 8 NeuronCores
(weights/scales replicated, no collectives). Per core (M=1024, K=N=4096):
  - x shard cast to bf16, transposed into a resident SBUF x^T via PE
    matmuls against identity (cheap: 3% of PE time, and it runs during
    the load-bound prologue)
  - per 512-wide out-feature tile: w_eff = sign*scale built in bf16 with
    per-partition tensor_scalar ops split 3:1 between DVE and ACT, then
    transposed on the PE against identity; the PSUM->SBUF evacuations
    alternate between DVE and ACT so neither engine saturates (the
    baseline put all of them on DVE, which was the second bottleneck)
  - w^T build chunks are emitted interleaved into the consuming m-loop so
    the in-order DVE queue never delays accumulator evacuations
  - PE main matmuls: 128x128x512 bf16, K accumulated in PSUM
  - PSUM evacuated to bf16 by DVE, stored as bf16, host casts to fp32
  (A DMA-xbar-transpose variant was tried and is numerically correct with
  a drain-based sync, but the xbar ucode's completion semaphores stall
  ~19us per transfer on HW, so the PE transpose wins.)

DMA queues: w loads ride the gpsimd (SWDGE) queue, x loads + y stores the
sync queue, xbar transposes the scalar queue.
"""

import sys

for _p in ("/opt/trn_rl_repo", "/opt/pypackages"):
    if _p not in sys.path:
        sys.path.append(_p)

import numpy as np

import concourse.bacc as bacc
import concourse.mybir as mybir
from concourse.bass import _add_dep_helper
from concourse.bass_utils import run_bass_kernel_spmd
from concourse.masks import make_identity
from concourse.tile import TileContext

N_CORES = 8
B, S, IN_F, OUT_F = 4, 2048, 4096, 4096
GROUP = 128
M_FULL = B * S  # 8192


def build_program(M=M_FULL // N_CORES, K=IN_F, N=OUT_F, n_tile=512, ld=1024):
    """Emit the per-core Bass program (SPMD: same program on all cores)."""
    P = 128
    KT = K // P            # k chunks of 128 (contraction)
    MT = M // P            # m tiles of 128
    NT = N // n_tile       # out-feature tiles
    NSUB = n_tile // P     # 128-row o sub-blocks per o tile
    LC = K // ld           # load chunks per 128-row block
    LG = ld // P           # 128-col groups per load chunk
    G = K // GROUP         # scale groups along in_features
    NB = N // P            # o blocks of 128
    bf16 = mybir.dt.bfloat16
    f32 = mybir.dt.float32

    nc = bacc.Bacc("TRN2", target_bir_lowering=False, debug=False)
    x_d = nc.dram_tensor("x", [M, K], f32, kind="ExternalInput").ap()
    w_d = nc.dram_tensor("sw", [N, K], f32, kind="ExternalInput").ap()
    sc_d = nc.dram_tensor("sc", [N, G], f32, kind="ExternalInput").ap()
    y_d = nc.dram_tensor("y", [M, N], bf16, kind="ExternalOutput").ap()

    with TileContext(nc) as tc:
        with (
            tc.tile_pool(name="consts", bufs=1) as consts,
            tc.tile_pool(name="xt_pool", bufs=1) as xt_pool,
            tc.tile_pool(name="wt_pool", bufs=3) as wt_pool,
            tc.tile_pool(name="wload", bufs=3) as wload_pool,
            tc.tile_pool(name="wstage", bufs=6) as wstage_pool,
            tc.tile_pool(name="xload", bufs=2) as xload_pool,
            tc.tile_pool(name="xstage", bufs=2) as xstage_pool,
            tc.tile_pool(name="ysb", bufs=4) as y_pool,
            tc.tile_pool(name="acc", bufs=5, space="PSUM") as psum_a,
            tc.tile_pool(name="ptr", bufs=3, space="PSUM") as psum_t,
        ):
            ident = consts.tile([P, P], bf16)
            make_identity(nc, ident)

            # scales resident: sc_sb[p, ob*G + g] = scales[ob*128 + p, g]
            sc_sb = consts.tile([P, NB * G], f32)
            nc.sync.dma_start(
                out=sc_sb.rearrange("p (ob g) -> p ob g", ob=NB),
                in_=sc_d.rearrange("(ob p) g -> p ob g", p=P),
            )

            # work splitters: scale ops 3:1 DVE:ACT, casts/evacs alternate
            st = [0, 0]

            def veng_scale(out, in_, scalar):
                i = st[0]
                st[0] += 1
                if i % 4 < 3:
                    nc.vector.tensor_scalar_mul(out=out, in0=in_, scalar1=scalar)
                else:
                    nc.scalar.mul(out, in_, scalar)

            def veng_copy(out, in_):
                i = st[1]
                st[1] += 1
                if i % 2 == 0:
                    nc.vector.tensor_copy(out=out, in_=in_)
                else:
                    nc.scalar.copy(out=out, in_=in_)

            # ---- x^T resident (bf16): xT[p, k, m] ----
            xT = xt_pool.tile([P, KT, M], bf16)

            def emit_x_chunk(mt, lc):
                xin = xload_pool.tile([P, ld], f32, tag="xload")
                nc.sync.dma_start(
                    out=xin,
                    in_=x_d[mt * P:(mt + 1) * P, lc * ld:(lc + 1) * ld],
                )
                xst = xstage_pool.tile([P, ld], bf16, tag="xstage")
                veng_copy(xst, xin)
                # PE transpose against identity, 4 blocks per PSUM tile
                for h in range(LG // 4):
                    pt = psum_t.tile([P, 512], f32, tag="pt")
                    for g in range(4):
                        c = h * 4 + g
                        nc.tensor.matmul(
                            pt[:, g * P:(g + 1) * P],
                            xst[:, c * P:(c + 1) * P],
                            ident,
                            start=True,
                            stop=True,
                        )
                    pt_v = pt.rearrange("p (g c) -> p g c", g=4)
                    veng_copy(
                        xT[:, lc * LG + h * 4:lc * LG + h * 4 + 4,
                           mt * P:(mt + 1) * P],
                        pt_v,
                    )

            # ---- w_eff^T build for one 512-wide o tile (PE transposes) ----
            def emit_build_chunk(wT, ot, c, load_eng=None):
                """One (j, lc) chunk of the wT build: load, scale, transpose."""
                j, lc = divmod(c, LC)
                ob = ot * NSUB + j
                win = wload_pool.tile([P, ld], f32, tag="wload")
                (load_eng or nc.gpsimd).dma_start(
                    out=win,
                    in_=w_d[ob * P:(ob + 1) * P, lc * ld:(lc + 1) * ld],
                )
                wst = wstage_pool.tile([P, ld], bf16, tag="wstage")
                for g in range(LG):
                    gk = lc * LG + g
                    veng_scale(
                        wst[:, g * P:(g + 1) * P],
                        win[:, g * P:(g + 1) * P],
                        sc_sb[:, ob * G + gk:ob * G + gk + 1],
                    )
                for h in range(LG // 4):
                    pt = psum_t.tile([P, 512], f32, tag="pt")
                    for g in range(4):
                        cc = h * 4 + g
                        nc.tensor.matmul(
                            pt[:, g * P:(g + 1) * P],
                            wst[:, cc * P:(cc + 1) * P],
                            ident,
                            start=True,
                            stop=True,
                        )
                    pt_v = pt.rearrange("p (g c) -> p g c", g=4)
                    veng_copy(
                        wT[:, lc * LG + h * 4:lc * LG + h * 4 + 4,
                           j * P:(j + 1) * P],
                        pt_v,
                    )

            NCH = NSUB * LC  # build chunks per o tile (16)

            def build_wT_all(ot, split_loads=False):
                wT = wt_pool.tile([P, KT, n_tile], bf16, tag="wt")
                for c in range(NCH):
                    eng = (nc.scalar if (split_loads and c % 2) else None)
                    emit_build_chunk(wT, ot, c, load_eng=eng)
                return wT

            def emit_m_tile(ot, wT_v, mt):
                acc = psum_a.tile([P, n_tile], f32, tag="acc")
                for k in range(KT):
                    nc.tensor.matmul(
                        acc,
                        xT[:, k, mt * P:(mt + 1) * P],
                        wT_v[:, k],
                        start=(k == 0),
                        stop=(k == KT - 1),
                    )
                ysb = y_pool.tile([P, n_tile], bf16, tag="ysb")
                nc.vector.tensor_copy(out=ysb, in_=acc)
                nc.sync.dma_start(
                    out=y_d[mt * P:(mt + 1) * P,
                            ot * n_tile:(ot + 1) * n_tile],
                    in_=ysb,
                )

            # ---- prologue: first two wT builds, then x phase with o-tile 0
            #      matmuls folded in so the PE never idles while x loads ----
            wT_cur = build_wT_all(0, split_loads=True)
            wT_nxt = build_wT_all(1) if NT > 1 else None
            wT_pre2 = (wt_pool.tile([P, KT, n_tile], bf16, tag="wt",
                                    name="wT_pre2")
                       if NT > 2 else None)
            for mt in range(MT):
                for lc in range(LC):
                    emit_x_chunk(mt, lc)
                emit_m_tile(0, wT_cur, mt)
                if wT_pre2 is not None:
                    for c in range(2 * mt, 2 * mt + 2):
                        emit_build_chunk(wT_pre2, 2, c)

            # ---- main loop over remaining o tiles, build for ot+2
            #      interleaved so DVE evacs never sit behind scale bursts ----
            wT_cur, wT_nxt = wT_nxt, wT_pre2
            for ot in range(1, NT):
                build = ot + 2 < NT
                wT_pre = (wt_pool.tile([P, KT, n_tile], bf16, tag="wt",
                                       name="wT_pre")
                          if build else None)
                for mt in range(MT):
                    emit_m_tile(ot, wT_cur, mt)
                    if build:
                        for c in range(2 * mt, 2 * mt + 2):
                            emit_build_chunk(wT_pre, ot + 2, c)
                wT_cur = wT_nxt
                wT_nxt = wT_pre

    nc.compile()
    return nc


_nc_cache = {}


def _get_nc(key, **kw):
    if key not in _nc_cache:
        _nc_cache[key] = build_program(**kw)
    return _nc_cache[key]


def _make_in_maps(x, sign_weights, scales):
    M_SH = M_FULL // N_CORES
    xf = np.ascontiguousarray(x.reshape(M_FULL, IN_F).astype(np.float32, copy=False))
    sw = np.ascontiguousarray(sign_weights.astype(np.float32, copy=False))
    sc = np.ascontiguousarray(scales.reshape(OUT_F, IN_F // GROUP))
    return [
        {"x": xf[c * M_SH:(c + 1) * M_SH], "sw": sw, "sc": sc}
        for c in range(N_CORES)
    ]


def _assemble(results):
    y = np.concatenate([results[c]["y"] for c in range(N_CORES)], axis=0)
    return y.reshape(B, S, OUT_F).astype(np.float32)


def kernel(x: np.ndarray, sign_weights: np.ndarray, scales: np.ndarray) -> np.ndarray:
    nc = _get_nc("full")
    in_maps = _make_in_maps(x, sign_weights, scales)
    res = run_bass_kernel_spmd(nc, in_maps, core_ids=list(range(N_CORES)))
    return _assemble(res.results)


# revision 25
# speedup vs baseline: 1.0037x; 1.0037x over previous
"""STEBitLinear Trainium2 kernel.

y[b,s,o] = sum_i x[b,s,i] * sign(w[o,i]) * scale[o, i//128]

Strategy: data-parallel over the flattened (b,s) dim across 8 NeuronCores
(weights/scales replicated, no collectives). Per core (M=1024, K=N=4096):
  - x shard cast to bf16, transposed into a resident SBUF x^T via PE
    matmuls against identity (cheap: 3% of PE time, and it runs during
    the load-bound prologue)
  - per 512-wide out-feature tile: w_eff = sign*scale built in bf16 with
    per-partition tensor_scalar ops split 3:1 between DVE and ACT, then
    transposed on the PE against identity; the PSUM->SBUF evacuations
    alternate between DVE and ACT so neither engine saturates (the
    baseline put all of them on DVE, which was the second bottleneck)
  - w^T build chunks are emitted interleaved into the consuming m-loop so
    the in-order DVE queue never delays accumulator evacuations
  - PE main matmuls: 128x128x512 bf16, K accumulated in PSUM
  - PSUM evacuated to bf16 by DVE, stored as bf16, host casts to fp32
  (A DMA-xbar-transpose variant was tried and is numerically correct with
  a drain-based sync, but the xbar ucode's completion semaphores stall
  ~19us per transfer on HW, so the PE transpose wins.)

DMA queues: w loads ride the gpsimd (SWDGE) queue, x loads + y stores the
sync queue, xbar transposes the scalar queue.
"""

import sys

for _p in ("/opt/trn_rl_repo", "/opt/pypackages"):
    if _p not in sys.path:
        sys.path.append(_p)

import numpy as np

import concourse.bacc as bacc
import concourse.mybir as mybir
from concourse.bass import _add_dep_helper
from concourse.bass_utils import run_bass_kernel_spmd
from concourse.masks import make_identity
from concourse.tile import TileContext

N_CORES = 8
B, S, IN_F, OUT_F = 4, 2048, 4096, 4096
GROUP = 128
M_FULL = B * S  # 8192


def build_program(M=M_FULL // N_CORES, K=IN_F, N=OUT_F, n_tile=512, ld=1024):
    """Emit the per-core Bass program (SPMD: same program on all cores)."""
    P = 128
    KT = K // P            # k chunks of 128 (contraction)
    MT = M // P            # m tiles of 128
    NT = N // n_tile       # out-feature tiles
    NSUB = n_tile // P     # 128-row o sub-blocks per o tile
    LC = K // ld           # load chunks per 128-row block
    LG = ld // P           # 128-col groups per load chunk
    G = K // GROUP         # scale groups along in_features
    NB = N // P            # o blocks of 128
    bf16 = mybir.dt.bfloat16
    f32 = mybir.dt.float32

    nc = bacc.Bacc("TRN2", target_bir_lowering=False, debug=False)
    x_d = nc.dram_tensor("x", [M, K], f32, kind="ExternalInput").ap()
    w_d = nc.dram_tensor("sw", [N, K], f32, kind="ExternalInput").ap()
    sc_d = nc.dram_tensor("sc", [N, G], f32, kind="ExternalInput").ap()
    y_d = nc.dram_tensor("y", [M, N], bf16, kind="ExternalOutput").ap()

    with TileContext(nc) as tc:
        with (
            tc.tile_pool(name="consts", bufs=1) as consts,
            tc.tile_pool(name="xt_pool", bufs=1) as xt_pool,
            tc.tile_pool(name="wt_pool", bufs=3) as wt_pool,
            tc.tile_pool(name="wload", bufs=3) as wload_pool,
            tc.tile_pool(name="wstage", bufs=6) as wstage_pool,
            tc.tile_pool(name="xload", bufs=2) as xload_pool,
            tc.tile_pool(name="xstage", bufs=2) as xstage_pool,
            tc.tile_pool(name="ysb", bufs=4) as y_pool,
            tc.tile_pool(name="acc", bufs=5, space="PSUM") as psum_a,
            tc.tile_pool(name="ptr", bufs=3, space="PSUM") as psum_t,
        ):
            ident = consts.tile([P, P], bf16)
            make_identity(nc, ident)

            # scales resident: sc_sb[p, ob*G + g] = scales[ob*128 + p, g]
            sc_sb = consts.tile([P, NB * G], f32)
            nc.sync.dma_start(
                out=sc_sb.rearrange("p (ob g) -> p ob g", ob=NB),
                in_=sc_d.rearrange("(ob p) g -> p ob g", p=P),
            )

            # work splitters: scale ops 3:1 DVE:ACT, casts/evacs alternate
            st = [0, 0]

            def veng_scale(out, in_, scalar):
                i = st[0]
                st[0] += 1
                if i % 4 < 3:
                    nc.vector.tensor_scalar_mul(out=out, in0=in_, scalar1=scalar)
                else:
                    nc.scalar.mul(out, in_, scalar)

            def veng_copy(out, in_):
                i = st[1]
                st[1] += 1
                if i % 2 == 0:
                    nc.vector.tensor_copy(out=out, in_=in_)
                else:
                    nc.scalar.copy(out=out, in_=in_)

            # ---- x^T resident (bf16): xT[p, k, m] ----
            xT = xt_pool.tile([P, KT, M], bf16)

            def emit_x_chunk(mt, lc):
                xin = xload_pool.tile([P, ld], f32, tag="xload")
                xeng = nc.sync if lc % 2 == 0 else nc.scalar
                xeng.dma_start(
                    out=xin,
                    in_=x_d[mt * P:(mt + 1) * P, lc * ld:(lc + 1) * ld],
                )
                xst = xstage_pool.tile([P, ld], bf16, tag="xstage")
                veng_copy(xst, xin)
                # PE transpose against identity, 4 blocks per PSUM tile
                for h in range(LG // 4):
                    pt = psum_t.tile([P, 512], f32, tag="pt")
                    for g in range(4):
                        c = h * 4 + g
                        nc.tensor.matmul(
                            pt[:, g * P:(g + 1) * P],
                            xst[:, c * P:(c + 1) * P],
                            ident,
                            start=True,
                            stop=True,
                        )
                    pt_v = pt.rearrange("p (g c) -> p g c", g=4)
                    veng_copy(
                        xT[:, lc * LG + h * 4:lc * LG + h * 4 + 4,
                           mt * P:(mt + 1) * P],
                        pt_v,
                    )

            # ---- w_eff^T build for one 512-wide o tile (PE transposes) ----
            def emit_build_chunk(wT, ot, c, load_eng=None):
                """One (j, lc) chunk of the wT build: load, scale, transpose."""
                j, lc = divmod(c, LC)
                ob = ot * NSUB + j
                win = wload_pool.tile([P, ld], f32, tag="wload")
                (load_eng or nc.gpsimd).dma_start(
                    out=win,
                    in_=w_d[ob * P:(ob + 1) * P, lc * ld:(lc + 1) * ld],
                )
                wst = wstage_pool.tile([P, ld], bf16, tag="wstage")
                for g in range(LG):
                    gk = lc * LG + g
                    veng_scale(
                        wst[:, g * P:(g + 1) * P],
                        win[:, g * P:(g + 1) * P],
                        sc_sb[:, ob * G + gk:ob * G + gk + 1],
                    )
                for h in range(LG // 4):
                    pt = psum_t.tile([P, 512], f32, tag="pt")
                    for g in range(4):
                        cc = h * 4 + g
                        nc.tensor.matmul(
                            pt[:, g * P:(g + 1) * P],
                            wst[:, cc * P:(cc + 1) * P],
                            ident,
                            start=True,
                            stop=True,
                        )
                    pt_v = pt.rearrange("p (g c) -> p g c", g=4)
                    veng_copy(
                        wT[:, lc * LG + h * 4:lc * LG + h * 4 + 4,
                           j * P:(j + 1) * P],
                        pt_v,
                    )

            NCH = NSUB * LC  # build chunks per o tile (16)

            def build_wT_all(ot, split_loads=False):
                wT = wt_pool.tile([P, KT, n_tile], bf16, tag="wt")
                for c in range(NCH):
                    eng = (nc.scalar if (split_loads and c % 2) else None)
                    emit_build_chunk(wT, ot, c, load_eng=eng)
                return wT

            def emit_m_tile(ot, wT_v, mt):
                acc = psum_a.tile([P, n_tile], f32, tag="acc")
                for k in range(KT):
                    nc.tensor.matmul(
                        acc,
                        xT[:, k, mt * P:(mt + 1) * P],
                        wT_v[:, k],
                        start=(k == 0),
                        stop=(k == KT - 1),
                    )
                ysb = y_pool.tile([P, n_tile], bf16, tag="ysb")
                nc.vector.tensor_copy(out=ysb, in_=acc)
                nc.sync.dma_start(
                    out=y_d[mt * P:(mt + 1) * P,
                            ot * n_tile:(ot + 1) * n_tile],
                    in_=ysb,
                )

            # ---- prologue: first two wT builds, then x phase with o-tile 0
            #      matmuls folded in so the PE never idles while x loads ----
            # first two x m-tiles ahead of build 0 so the DVE serves the
            # x casts (which gate the PE's first work) before scale bursts
            for mt in range(2):
                for lc in range(LC):
                    emit_x_chunk(mt, lc)
            wT_cur = build_wT_all(0)
            wT_nxt = build_wT_all(1) if NT > 1 else None
            wT_pre2 = (wt_pool.tile([P, KT, n_tile], bf16, tag="wt",
                                    name="wT_pre2")
                       if NT > 2 else None)
            for mt in range(MT):
                if mt >= 2:
                    for lc in range(LC):
                        emit_x_chunk(mt, lc)
                emit_m_tile(0, wT_cur, mt)
                if wT_pre2 is not None:
                    for c in range(2 * mt, 2 * mt + 2):
                        emit_build_chunk(wT_pre2, 2, c)

            # ---- main loop over remaining o tiles, build for ot+2
            #      interleaved so DVE evacs never sit behind scale bursts ----
            wT_cur, wT_nxt = wT_nxt, wT_pre2
            for ot in range(1, NT):
                build = ot + 2 < NT
                wT_pre = (wt_pool.tile([P, KT, n_tile], bf16, tag="wt",
                                       name="wT_pre")
                          if build else None)
                for mt in range(MT):
                    emit_m_tile(ot, wT_cur, mt)
                    if build:
                        for c in range(2 * mt, 2 * mt + 2):
                            emit_build_chunk(wT_pre, ot + 2, c)
                wT_cur = wT_nxt
                wT_nxt = wT_pre

    nc.compile()
    return nc


_nc_cache = {}


def _get_nc(key, **kw):
    if key not in _nc_cache:
        _nc_cache[key] = build_program(**kw)
    return _nc_cache[key]


def _make_in_maps(x, sign_weights, scales):
    M_SH = M_FULL // N_CORES
    xf = np.ascontiguousarray(x.reshape(M_FULL, IN_F).astype(np.float32, copy=False))
    sw = np.ascontiguousarray(sign_weights.astype(np.float32, copy=False))
    sc = np.ascontiguousarray(scales.reshape(OUT_F, IN_F // GROUP))
    return [
        {"x": xf[c * M_SH:(c + 1) * M_SH], "sw": sw, "sc": sc}
        for c in range(N_CORES)
    ]


def _assemble(results):
    y = np.concatenate([results[c]["y"] for c in range(N_CORES)], axis=0)
    return y.reshape(B, S, OUT_F).astype(np.float32)


def kernel(x: np.ndarray, sign_weights: np.ndarray, scales: np.ndarray) -> np.ndarray:
    nc = _get_nc("full")
    in_maps = _make_in_maps(x, sign_weights, scales)
    res = run_bass_kernel_spmd(nc, in_maps, core_ids=list(range(N_CORES)))
    return _assemble(res.results)


# revision 26
# speedup vs baseline: 1.0216x; 1.0178x over previous
"""STEBitLinear Trainium2 kernel.

y[b,s,o] = sum_i x[b,s,i] * sign(w[o,i]) * scale[o, i//128]

Strategy: data-parallel over the flattened (b,s) dim across 8 NeuronCores
(weights/scales replicated, no collectives). Per core:
  - cast x shard to bf16 and transpose it into a resident SBUF x^T
  - per 512-wide out-feature tile: build w_eff^T = (sign*scale)^T in bf16
    (fused cast+scale via per-partition tensor_scalar, then transpose)
  - 128x128x512 bf16 matmuls accumulating over K=4096 in PSUM (fp32)

All transposes are NORMAL bf16 matmuls against a 128x128 identity
(out = chunk.T @ I): unlike PE transpose-mode these run at warm-matmul
speed and keep the HAM clock gate engaged. The o-tile pipeline is
software-pipelined at emission: the w^T build for tile t+1 is emitted
before tile t's matmul loop, so its PE transposes slot in right after
tile t's matmuls and its DVE scale ops run during them. PSUM result
evacuation runs on the otherwise-idle Scalar (ACT) engine so it never
head-of-line blocks DVE's scale pipeline.
"""

import sys

for _p in ("/opt/trn_rl_repo", "/opt/pypackages"):
    if _p not in sys.path:
        sys.path.append(_p)

import numpy as np

import concourse.bacc as bacc
import concourse.mybir as mybir
from concourse.bass_utils import run_bass_kernel_spmd
from concourse.masks import make_identity
from concourse.tile import TileContext

N_CORES = 8
B, S, IN_F, OUT_F = 4, 2048, 4096, 4096
GROUP = 128
M_FULL = B * S  # 8192


def build_program(M=M_FULL // N_CORES, K=IN_F, N=OUT_F, n_tile=512, ld=1024):
    """Emit the per-core Bass program (SPMD: same program on all cores)."""
    P = 128
    KT = K // P            # k tiles (contraction, partition dim)
    MT = M // P            # m tiles
    NT = N // n_tile       # out-feature tiles
    NSUB = n_tile // P     # 128-wide o sub-blocks per o tile
    LC = K // ld           # load chunks per row-block
    LG = ld // P           # 128-wide groups per load chunk
    G = K // GROUP         # scale groups along in_features
    NB = N // P            # o blocks of 128
    bf16 = mybir.dt.bfloat16
    f32 = mybir.dt.float32

    nc = bacc.Bacc("TRN2", target_bir_lowering=False, debug=False)
    x_d = nc.dram_tensor("x", [M, K], f32, kind="ExternalInput").ap()
    w_d = nc.dram_tensor("sw", [N, K], f32, kind="ExternalInput").ap()
    sc_d = nc.dram_tensor("sc", [N, G], f32, kind="ExternalInput").ap()
    y_d = nc.dram_tensor("y", [M, N], f32, kind="ExternalOutput").ap()

    with TileContext(nc) as tc:
        with (
            tc.tile_pool(name="consts", bufs=1) as consts,
            tc.tile_pool(name="xt_pool", bufs=1) as xt_pool,
            tc.tile_pool(name="wt_pool", bufs=2) as wt_pool,
            tc.tile_pool(name="load", bufs=3) as load_pool,
            tc.tile_pool(name="stage", bufs=4) as stage_pool,
            tc.tile_pool(name="ysb", bufs=2) as y_pool,
            tc.tile_pool(name="pst", bufs=5, space="PSUM") as psum_t,
            tc.tile_pool(name="psa", bufs=3, space="PSUM") as psum_a,
        ):
            ident = consts.tile([P, P], bf16)
            make_identity(nc, ident)

            # scales resident: sc_sb[p, ob*G + g] = scales[ob*128 + p, g]
            # (gpsimd/SWDGE ring: keeps the HWDGE ring free for x/w loads)
            sc_sb = consts.tile([P, NB * G], f32)
            for ob in range(NB):
                nc.gpsimd.dma_start(
                    out=sc_sb[:, ob * G:(ob + 1) * G],
                    in_=sc_d[ob * P:(ob + 1) * P, :],
                )

            def mm_transpose(dst_v, src, k0, col0):
                """dst_v[:, k0+c, col0:col0+128] = src[:, c*128:(c+1)*128].T
                for c in range(LG), via normal matmuls against identity."""
                for h in range(LG // 4):
                    pt = psum_t.tile([P, 512], f32, tag="pt")
                    for g in range(4):
                        c = h * 4 + g
                        nc.tensor.matmul(
                            pt[:, g * P:(g + 1) * P],
                            src[:, c * P:(c + 1) * P],
                            ident,
                            start=True,
                            stop=True,
                        )
                    pt_v = pt.rearrange("p (g c) -> p g c", g=4)
                    nc.vector.tensor_copy(
                        out=dst_v[:, k0 + h * 4:k0 + h * 4 + 4, col0:col0 + P],
                        in_=pt_v,
                    )

            # ---- phase 0: x^T resident (bf16), [P, KT * M] ----
            xT = xt_pool.tile([P, KT * M], bf16)
            xT_v = xT.rearrange("p (k m) -> p k m", k=KT)
            for mt in range(MT):
                for lc in range(LC):
                    xin = load_pool.tile([P, ld], f32, tag="xload")
                    nc.sync.dma_start(
                        out=xin,
                        in_=x_d[mt * P:(mt + 1) * P, lc * ld:(lc + 1) * ld],
                    )
                    xbf = stage_pool.tile([P, ld], bf16, tag="xcast")
                    nc.vector.tensor_copy(out=xbf, in_=xin)
                    mm_transpose(xT_v, xbf, lc * LG, mt * P)

            # ---- main loop over out-feature tiles (software-pipelined) ----
            def build_wT(ot):
                """w_eff^T tiles for o tile `ot`: load, scale (DVE),
                transpose (PE), gather into a [P, KT * n_tile] bf16 tile."""
                wT = wt_pool.tile([P, KT * n_tile], bf16, tag="wt")
                wT_v = wT.rearrange("p (k o) -> p k o", k=KT)
                for j in range(NSUB):
                    ob = ot * NSUB + j
                    for lc in range(LC):
                        win = load_pool.tile([P, ld], f32, tag="wload")
                        nc.sync.dma_start(
                            out=win,
                            in_=w_d[ob * P:(ob + 1) * P, lc * ld:(lc + 1) * ld],
                        )
                        wst = stage_pool.tile([P, ld], bf16, tag="wstage")
                        for g in range(LG):
                            gk = lc * LG + g
                            nc.vector.tensor_scalar_mul(
                                out=wst[:, g * P:(g + 1) * P],
                                in0=win[:, g * P:(g + 1) * P],
                                scalar1=sc_sb[:, ob * G + gk:ob * G + gk + 1],
                            )
                        mm_transpose(wT_v, wst, lc * LG, j * P)
                return wT_v

            wT_cur = build_wT(0)
            wT_nxt = build_wT(1) if NT > 1 else None
            for ot in range(NT):
                wT_v = wT_cur
                for mt in range(MT):
                    acc = psum_a.tile([P, n_tile], f32, tag="acc")
                    for k in range(KT):
                        nc.tensor.matmul(
                            acc,
                            xT_v[:, k, mt * P:(mt + 1) * P],
                            wT_v[:, k],
                            start=(k == 0),
                            stop=(k == KT - 1),
                        )
                    ysb = y_pool.tile([P, n_tile], f32, tag="ysb")
                    nc.scalar.copy(out=ysb, in_=acc)
                    nc.sync.dma_start(
                        out=y_d[mt * P:(mt + 1) * P, ot * n_tile:(ot + 1) * n_tile],
                        in_=ysb,
                    )
                wT_cur = wT_nxt
                if ot + 2 < NT:
                    wT_nxt = build_wT(ot + 2)

    nc.compile()
    return nc


_nc_cache = {}


def _get_nc(key, **kw):
    if key not in _nc_cache:
        _nc_cache[key] = build_program(**kw)
    return _nc_cache[key]


def _make_in_maps(x, sign_weights, scales):
    M_SH = M_FULL // N_CORES
    xf = np.ascontiguousarray(x.reshape(M_FULL, IN_F).astype(np.float32, copy=False))
    sw = np.ascontiguousarray(sign_weights.astype(np.float32, copy=False))
    sc = np.ascontiguousarray(scales.reshape(OUT_F, IN_F // GROUP))
    return [
        {"x": xf[c * M_SH:(c + 1) * M_SH], "sw": sw, "sc": sc}
        for c in range(N_CORES)
    ]


def _assemble(results):
    y = np.concatenate([results[c]["y"] for c in range(N_CORES)], axis=0)
    return y.reshape(B, S, OUT_F)


def kernel(x: np.ndarray, sign_weights: np.ndarray, scales: np.ndarray) -> np.ndarray:
    nc = _get_nc("full")
    in_maps = _make_in_maps(x, sign_weights, scales)
    res = run_bass_kernel_spmd(nc, in_maps, core_ids=list(range(N_CORES)))
    return _assemble(res.results)

